# revision 27
# baseline (speedup 1.0000x reference)
"""GANet SGA kernel for Trainium2, 8 NeuronCores (SPMD).

Model (reference.py): 4-directional semi-global aggregation over a cost
volume x[1,32,48,64,128] with guidance g[1,640,64,128], elementwise max
over directions, BN+ReLU, 3x3x3 conv, BN, residual ReLU.

Sharding:
  - Vertical scans (over H): W-sharded, 16 cols/core, run first.
  - AllToAll reshards vertical results to H-shards (10 rows incl. 1-row
    halo each side) while horizontal scans (H-sharded, 10 rows/core) run.
  - Combine + BN1 (AllReduce stats) + conv (9 matmuls, K=96=3dz*32ci over
    a dz-replicated y3) + BN2 (AllReduce) + residual, all H-sharded.

Scan histories/communication are fp16 (recurrence is contracting, so the
~5e-4 rounding stays ~2e-3); accumulators, weights, stats are fp32.
Host does only slicing/layout transposes; all arithmetic on device.
"""

import sys
import numpy as np
from contextlib import ExitStack

try:
    import concourse.bass  # noqa: F401
except ImportError:
    sys.path.insert(0, "/opt/trn_rl_repo")

C, D, H, W = 32, 48, 64, 128
NCORES = 8
RPC = H // NCORES          # owned rows per core
RH = RPC + 2               # rows incl halo
CPC = W // NCORES          # cols per core
DQ = D // 4                # final-phase d-quotient
EPS_BN = 1e-5
EPS_L1 = 1e-12
NTOT = float(D * H * W)    # BN count (B=1)
NEG_INF = -3.0e38

_prog_cache = {}


def _build_program():
    import concourse.bass as bass
    import concourse.tile as tile
    from concourse import mybir

    FP = mybir.dt.float32
    FH = mybir.dt.float16
    ALU = mybir.AluOpType
    ACT = mybir.ActivationFunctionType
    AX = mybir.AxisListType

    nc = bass.Bass(num_devices=NCORES)

    # ---------------- DRAM I/O ----------------
    def inp(name, shape, dtype=FP):
        return nc.declare_dram_parameter(name, list(shape), dtype, isOutput=False)

    xh = inp("xh", (RH * C, W, D))          # (r,c) flat; horizontal-scan layout
    k1h = inp("k1h", (RH * C, W, 5))
    k2h = inp("k2h", (RH * C, W, 5))
    xv = inp("xv", (CPC * C, H, D))         # (col,c) flat; vertical-scan layout
    k3v = inp("k3v", (CPC * C, H, 5))
    k4v = inp("k4v", (CPC * C, H, 5))
    xr = inp("xr", (128, DQ, RPC, W))       # residual, (dr,c) partition layout
    wstk = inp("wstk", (96, 9, C), FH)      # conv weights (dz,ci) x (dy,dx) x co
    bn1g = inp("bn1g", (128, 1))            # gamma/beta replicated 4x to 128
    bn1b = inp("bn1b", (128, 1))
    bn2g = inp("bn2g", (128, 1))
    bn2b = inp("bn2b", (128, 1))
    selg0 = inp("selg0", (128, 128))        # BN aggregation masks, 4x-replicated
    selg1 = inp("selg1", (128, 128))        # columns so stats come out [128,2]
    selg2 = inp("selg2", (64, 128))

    out_d = nc.declare_dram_parameter("out", [128, DQ, RPC, W], FP, isOutput=True)

    # Internal DRAM
    a2a_in = nc.dram_tensor("a2a_in", [NCORES, CPC * C, RH, D], FH, kind="Internal")
    a2a_out = nc.dram_tensor("a2a_out", [NCORES, CPC * C, RH, D], FH, kind="Internal")
    conv_dram = nc.dram_tensor("conv_dram", [128, DQ, RPC, W], FP, kind="Internal")
    bn1_in = nc.dram_tensor("bn1_in", [128, 2], FP, kind="Internal")
    bn1_out = nc.dram_tensor("bn1_out", [128, 2], FP, kind="Internal", addr_space="Shared")
    bn2_in = nc.dram_tensor("bn2_in", [128, 2], FP, kind="Internal")
    bn2_out = nc.dram_tensor("bn2_out", [128, 2], FP, kind="Internal", addr_space="Shared")

    RG = [list(range(NCORES))]

    with tile.TileContext(nc) as tc, ExitStack() as top:
        pool_g = top.enter_context(tc.tile_pool(name="pg", bufs=1))   # small persistents
        psum_p = top.enter_context(tc.tile_pool(name="pp", bufs=1, space="PSUM"))

        # ---------- helpers ----------
        def normalize_k(pool, kt, P, L, tag):
            """L1-normalize kt [P, L, 5] along the 5 axis into a FRESH tile.

            Writing to a fresh tile (not in place) makes DVE the sole last
            writer, so downstream 3-src ops (1 sync-wait ISA slot on
            Pool/Act) depend on one semaphore only.
            """
            den = pool.tile([P, L], FP, name=f"den_{tag}", tag="den", bufs=3)
            nc.vector.tensor_reduce(
                out=den[:], in_=kt[:], axis=AX.X, op=ALU.add,
                apply_absolute_value=True,
            )
            nc.vector.tensor_scalar_max(den[:], den[:], EPS_L1)
            rec = pool.tile([P, L], FP, name=f"rec_{tag}", tag="rec", bufs=3)
            nc.vector.reciprocal(rec[:], den[:])
            ktn = pool.tile([P, L, 5], FP, name=f"kn_{tag}", tag=f"kn_{tag}")
            for j in range(5):
                nc.vector.tensor_mul(ktn[:, :, j], kt[:, :, j], rec[:])
            return ktn

        scan_counter = [0]

        def emit_scan(pool, x_g, kn_g, hist_g, L, reverse, P):
            """Sequential SGA scan along axis 'L' of x_g [P, L, D].

            hist_g [P, L, D] fp16 holds the full scan history (= output).
            Weight slices kn_g [P, L, 5] are position-indexed like x.
            """
            cid = scan_counter[0]
            scan_counter[0] += 1
            mxA = pool.tile([P, 1], FP, name=f"mxA{cid}", tag=f"mxA{cid}")
            mxB = pool.tile([P, 1], FP, name=f"mxB{cid}", tag=f"mxB{cid}")
            mxw = pool.tile([P, 1], FP, name=f"mxw{cid}", tag=f"mxw{cid}")
            # First-touch on V: collapses later RAW+WAR deps of the 3-src
            # ops (1 sync-wait ISA slot) onto a single DVE semaphore.
            nc.vector.memset(mxB[:], 0.0)
            nc.vector.memset(mxw[:], 0.0)
            # hist pad column (index D) stays 0: zero source for the d+1 tap
            nc.vector.memset(hist_g[:, :, D:D + 1], 0.0)
            tmps, accs = [], []
            for b in range(2):
                tt = pool.tile([P, D], FP, name=f"tmp{cid}_{b}", tag=f"tmp{cid}_{b}")
                nc.vector.memset(tt[:], 0.0)
                tmps.append(tt)
                ac = pool.tile([P, D], FP, name=f"acc{cid}_{b}",
                               tag=f"acc{cid}_{b}")
                nc.vector.memset(ac[:], 0.0)
                accs.append(ac)

            pos0 = L - 1 if reverse else 0
            nc.vector.tensor_copy(hist_g[:, pos0, 0:D], x_g[:, pos0, :])
            nc.vector.tensor_reduce(out=mxA[:], in_=hist_g[:, pos0, 0:D],
                                    axis=AX.X, op=ALU.max)

            mx_prev, mx_next = mxA, mxB
            for s in range(1, L):
                t = L - 1 - s if reverse else s
                tp = t + 1 if reverse else t - 1
                prev = hist_g[:, tp, 0:D]
                prevp = hist_g[:, tp, 1:D + 1]   # d+1 shift, zero-padded
                cur = hist_g[:, t, 0:D]
                acc = accs[s % 2][:]
                tmp = tmps[s % 2]
                k0 = kn_g[:, t, 0:1]
                k1 = kn_g[:, t, 1:2]
                k2 = kn_g[:, t, 2:3]
                k3 = kn_g[:, t, 3:4]
                k4 = kn_g[:, t, 4:5]
                # Engine split is FIXED. Walrus limits TensorScalarPtr/
                # Activation-format ops to ONE sync wait, so every 3-src op
                # is arranged to depend on a single semaphore: GP ops read
                # only DVE-written tiles; the lone V op consuming GP output
                # is the 2-src-format ttr.
                # mxw = w4 * max_d(prev)   [GP]
                nc.gpsimd.tensor_scalar_mul(mxw[:], mx_prev[:], k4)
                # tmp = prev_shifted * w3 + mxw   (d+1 tap + best-disp) [GP]
                nc.gpsimd.tensor_scalar(
                    out=tmp[:], in0=prevp, scalar1=k3, scalar2=mxw[:],
                    op0=ALU.mult, op1=ALU.add,
                )
                # acc = x_t * w0    [V; own-engine deps only]
                nc.vector.tensor_scalar_mul(acc, x_g[:, t, :], k0)
                # acc[1:] += prev[:-1] * w1   (d-1 tap)  [V; all-V deps]
                nc.vector.scalar_tensor_tensor(
                    out=acc[:, 1:D], in0=prev[:, 0:D - 1], scalar=k1,
                    in1=acc[:, 1:D], op0=ALU.mult, op1=ALU.add,
                )
                # acc += prev * w2 (center)   [V; all-V deps]
                nc.vector.scalar_tensor_tensor(
                    out=acc, in0=prev, scalar=k2, in1=acc,
                    op0=ALU.mult, op1=ALU.add,
                )
                # cur = tmp + acc ; mx_next = max_d(cur)   [V; waits Pool(tmp)]
                nc.vector.tensor_tensor_reduce(
                    out=cur, in0=tmp[:], in1=acc, scale=1.0, scalar=NEG_INF,
                    op0=ALU.add, op1=ALU.max, accum_out=mx_next[:],
                )
                mx_prev, mx_next = mx_next, mx_prev

        # =======================================================
        # Phase V: vertical scans (W-sharded), feed AllToAll
        # =======================================================
        zrow = pool_g.tile([128, 1, D], FH, name="zrow", tag="zrow")
        nc.vector.memset(zrow[:], 0.0)
        # Dummy first activation: absorbs the ACT table-load pseudo-inst so
        # later activations keep both sync-wait slots (walrus S3D3_AC limit).
        actwarm = pool_g.tile([1, 1], FP, name="actwarm", tag="actwarm")
        nc.vector.memset(actwarm[:], 0.0)
        nc.scalar.activation(actwarm[:], actwarm[:], ACT.Identity, scale=1.0)

        with tc.tile_pool(name="pv", bufs=1) as pv:
            for g in range(4):
                xv_g = pv.tile([128, H, D], FP, name=f"xv{g}", tag=f"xv{g}")
                nc.sync.dma_start(xv_g[:], xv[128 * g:128 * (g + 1), :, :])
                k3_g = pv.tile([128, H, 5], FP, name=f"k3{g}", tag=f"k3{g}")
                nc.sync.dma_start(k3_g[:], k3v[128 * g:128 * (g + 1), :, :])
                k4_g = pv.tile([128, H, 5], FP, name=f"k4{g}", tag=f"k4{g}")
                nc.sync.dma_start(k4_g[:], k4v[128 * g:128 * (g + 1), :, :])
                k3_n = normalize_k(pv, k3_g, 128, H, f"v3{g}")
                k4_n = normalize_k(pv, k4_g, 128, H, f"v4{g}")
                a3_g = pv.tile([128, H, D + 1], FH, name=f"a3{g}", tag=f"a3{g}")
                h4_g = pv.tile([128, H, D + 1], FH, name=f"h4{g}", tag=f"h4{g}")
                emit_scan(pv, xv_g, k3_n, a3_g, H, reverse=False, P=128)
                emit_scan(pv, xv_g, k4_n, h4_g, H, reverse=True, P=128)
                nc.vector.tensor_max(a3_g[:, :, 0:D], a3_g[:, :, 0:D],
                                     h4_g[:, :, 0:D])
                # send chunks (1-row halo each side; zero out-of-range rows)
                for j in range(NCORES):
                    h0 = 8 * j - 1
                    dst = a2a_in[j, 128 * g:128 * (g + 1), :, :]
                    if h0 < 0:
                        nc.sync.dma_start(dst[:, 1:RH, :], a3_g[:, 0:RH - 1, 0:D])
                        nc.sync.dma_start(dst[:, 0:1, :], zrow[:])
                    elif h0 + RH > H:
                        nc.sync.dma_start(dst[:, 0:RH - 1, :], a3_g[:, h0:H, 0:D])
                        nc.sync.dma_start(dst[:, RH - 1:RH, :], zrow[:])
                    else:
                        nc.sync.dma_start(dst, a3_g[:, h0:h0 + RH, 0:D])

        nc.gpsimd.collective_compute(
            "AllToAll", ALU.bypass, replica_groups=RG,
            ins=[a2a_in.ap().opt()], outs=[a2a_out.ap().opt()],
        )

        # =======================================================
        # Phase H: horizontal scans (H-sharded, 10 rows w/ halo)
        # Overlaps the AllToAll (no dependency).
        # =======================================================
        HG = [(0, 128), (128, 128), (256, 64)]  # (row-part offset, partitions)
        with ExitStack() as hs:
            ph = hs.enter_context(tc.tile_pool(name="ph", bufs=1))
            a1_tiles = []
            with tc.tile_pool(name="phx", bufs=1) as phx:
                for g, (p0, P) in enumerate(HG):
                    xh_g = phx.tile([P, W, D], FP, name=f"xh{g}", tag=f"xh{g}")
                    nc.sync.dma_start(xh_g[:], xh[p0:p0 + P, :, :])
                    k1_g = phx.tile([P, W, 5], FP, name=f"k1{g}", tag=f"k1{g}")
                    nc.sync.dma_start(k1_g[:], k1h[p0:p0 + P, :, :])
                    k2_g = phx.tile([P, W, 5], FP, name=f"k2{g}", tag=f"k2{g}")
                    nc.sync.dma_start(k2_g[:], k2h[p0:p0 + P, :, :])
                    k1_n = normalize_k(phx, k1_g, P, W, f"h1{g}")
                    k2_n = normalize_k(phx, k2_g, P, W, f"h2{g}")
                    a1_g = ph.tile([P, W, D + 1], FH, name=f"a1{g}", tag=f"a1{g}")
                    h2_g = ph.tile([P, W, D + 1], FH, name=f"h2{g}", tag=f"h2{g}")
                    emit_scan(phx, xh_g, k1_n, a1_g, W, reverse=False, P=P)
                    emit_scan(phx, xh_g, k2_n, h2_g, W, reverse=True, P=P)
                    nc.vector.tensor_max(a1_g[:, :, 0:D], a1_g[:, :, 0:D],
                                         h2_g[:, :, 0:D])
                    a1_tiles.append(a1_g)

            # ===================================================
            # Combine + BN1 stats
            # ===================================================
            pcv = hs.enter_context(tc.tile_pool(name="pcv", bufs=1))
            bnp = psum_p.tile([128, 2], FP, name="bnp", tag="bnp")
            sel_tiles = []
            for nm, ap_, P in (("s0", selg0, 128), ("s1", selg1, 128), ("s2", selg2, 64)):
                selt = pool_g.tile([P, 128], FP, name=f"sel{nm}", tag=f"sel{nm}")
                nc.sync.dma_start(selt[:], ap_[:])
                # launder through V so PE matmuls depend on DVE only
                sell = pool_g.tile([P, 128], FP, name=f"sell{nm}", tag=f"sell{nm}")
                nc.vector.tensor_copy(sell[:], selt[:])
                sel_tiles.append(sell)

            for g, (p0, P) in enumerate(HG):
                a1_g = a1_tiles[g]
                # padded like the hist tiles so the ttrs below see flat
                # contiguous [P, W*(D+1)] APs (ISA encoding limit); pads are
                # zero on both sides and contribute nothing to the stats
                a34_g = pcv.tile([P, W, D + 1], FH, name=f"a34_{g}", tag="a34",
                                 bufs=2)
                nc.vector.memset(a34_g[:, :, D:D + 1], 0.0)
                r0, nr = p0 // C, P // C
                for s in range(NCORES):
                    for rl in range(nr):
                        src = a2a_out[s, :, r0 + rl, :] \
                            .rearrange("(col c) d -> c col d", c=C)
                        nc.sync.dma_start(
                            a34_g[C * rl:C * (rl + 1), CPC * s:CPC * (s + 1), 0:D],
                            src)
                s12_g = pool_g.tile([P, 2], FP, name=f"bns{g}", tag=f"bns{g}")
                a1f = a1_g[:].rearrange("p w d -> p (w d)")
                a34f = a34_g[:].rearrange("p w d -> p (w d)")
                # ymax = max(a12, a34) in place, fused sum accumulate
                nc.vector.tensor_tensor_reduce(
                    out=a1f, in0=a1f, in1=a34f, scale=1.0, scalar=0.0,
                    op0=ALU.max, op1=ALU.add, accum_out=s12_g[:, 0:1],
                )
                # sum of squares on V (keeps the stats matmul all-DVE-dep;
                # scratch output reuses a34_g)
                nc.vector.tensor_tensor_reduce(
                    out=a34f, in0=a1f, in1=a1f, scale=1.0, scalar=0.0,
                    op0=ALU.mult, op1=ALU.add, accum_out=s12_g[:, 1:2],
                )
                nc.tensor.matmul(bnp[:], lhsT=sel_tiles[g][:], rhs=s12_g[:],
                                 start=(g == 0), stop=(g == 2))

            bnst = pool_g.tile([128, 2], FP, name="bnst", tag="bnst")
            nc.vector.tensor_copy(bnst[:], bnp[:])
            nc.sync.dma_start(bn1_in[:], bnst[:])
            nc.gpsimd.collective_compute(
                "AllReduce", ALU.add, replica_groups=RG,
                ins=[bn1_in.ap().opt()], outs=[bn1_out.ap().opt()],
            )

            def bn_scale_bias(bn_out_dram, gam, bet, tag):
                """([128,1] scale, [128,1] bias) from AllReduce'd (sum, sumsq).

                All tiles are [128,*] (stats already 4x-replicated); DMA'd
                inputs are laundered through V copies first.
                """
                st0 = pool_g.tile([128, 2], FP, name=f"bnr0{tag}", tag=f"bnr0{tag}")
                nc.sync.dma_start(st0[:], bn_out_dram[:])
                gt0 = pool_g.tile([128, 1], FP, name=f"bng0{tag}", tag=f"bng0{tag}")
                nc.sync.dma_start(gt0[:], gam[:])
                bt0 = pool_g.tile([128, 1], FP, name=f"bnb0{tag}", tag=f"bnb0{tag}")
                nc.sync.dma_start(bt0[:], bet[:])
                st = pool_g.tile([128, 2], FP, name=f"bnr{tag}", tag=f"bnr{tag}")
                nc.vector.tensor_copy(st[:], st0[:])
                gt = pool_g.tile([128, 1], FP, name=f"bng{tag}", tag=f"bng{tag}")
                nc.vector.tensor_copy(gt[:], gt0[:])
                bt = pool_g.tile([128, 1], FP, name=f"bnb{tag}", tag=f"bnb{tag}")
                nc.vector.tensor_copy(bt[:], bt0[:])
                mean = pool_g.tile([128, 1], FP, name=f"mean{tag}", tag=f"mean{tag}")
                nc.vector.tensor_scalar_mul(mean[:], st[:, 0:1], 1.0 / NTOT)
                var = pool_g.tile([128, 1], FP, name=f"var{tag}", tag=f"var{tag}")
                nc.vector.tensor_scalar_mul(var[:], st[:, 1:2], 1.0 / NTOT)
                msq = pool_g.tile([128, 1], FP, name=f"msq{tag}", tag=f"msq{tag}")
                nc.vector.tensor_mul(msq[:], mean[:], mean[:])
                nc.vector.tensor_tensor(out=var[:], in0=var[:], in1=msq[:],
                                        op=ALU.subtract)
                nc.vector.tensor_scalar_add(var[:], var[:], EPS_BN)
                sd = pool_g.tile([128, 1], FP, name=f"sd{tag}", tag=f"sd{tag}")
                nc.scalar.activation(sd[:], var[:], ACT.Sqrt)
                rs = pool_g.tile([128, 1], FP, name=f"rs{tag}", tag=f"rs{tag}")
                nc.vector.reciprocal(rs[:], sd[:])
                sc = pool_g.tile([128, 1], FP, name=f"sc{tag}", tag=f"sc{tag}")
                nc.vector.tensor_mul(sc[:], rs[:], gt[:])
                bi = pool_g.tile([128, 1], FP, name=f"bi{tag}", tag=f"bi{tag}")
                nc.vector.tensor_mul(bi[:], mean[:], sc[:])
                nc.vector.tensor_tensor(out=bi[:], in0=bt[:], in1=bi[:],
                                        op=ALU.subtract)
                return sc, bi

            sc1, bi1 = bn_scale_bias(bn1_out, bn1g, bn1b, "1")
            # y = relu(scale*ymax + bias), in place (halo rows included)
            for g, (p0, P) in enumerate(HG):
                nc.scalar.activation(a1_tiles[g][:, :, 0:D], a1_tiles[g][:, :, 0:D],
                                     ACT.Relu, bias=bi1[0:P, :], scale=sc1[0:P, :])

            # ===================================================
            # Conv 3x3x3: 9 matmuls (K=96: 3dz x 32ci), D in 3 chunks
            # ===================================================
            wst0 = pool_g.tile([96, 9, C], FH, name="wst0", tag="wst0")
            nc.sync.dma_start(wst0[:], wstk[:])
            wst = pool_g.tile([96, 9, C], FH, name="wst", tag="wst")
            nc.vector.tensor_copy(wst[:], wst0[:])   # launder for PE

            DC = 16
            WP = W + 2
            taps = [(a, b) for a in range(3) for b in range(3)]
            for ch in range(3):
                d0 = DC * ch
                # y3[(dz,ci), r, wp, dp] = ypad[ci, d0+dp+dz-1, r, wp-1]
                y3 = pcv.tile([96, RH, WP, DC], FH, name=f"y3_{ch}", tag="y3", bufs=1)
                nc.vector.memset(y3[:, :, 0:1, :], 0.0)
                nc.vector.memset(y3[:, :, WP - 1:WP, :], 0.0)
                for dz in range(3):
                    dp_lo = max(0, 1 - dz - d0)
                    dp_hi = min(DC - 1, 48 - d0 - dz)  # inclusive
                    if dp_lo > 0:
                        nc.vector.memset(y3[32 * dz:32 * (dz + 1), :, :, 0:dp_lo], 0.0)
                    if dp_hi < DC - 1:
                        nc.vector.memset(
                            y3[32 * dz:32 * (dz + 1), :, :, dp_hi + 1:DC], 0.0)
                    dlo = d0 + dp_lo + dz - 1
                    n_d = dp_hi - dp_lo + 1
                    for g, (p0, P) in enumerate(HG):
                        r0, nr = p0 // C, P // C
                        for rl in range(nr):
                            src = a1_tiles[g][C * rl:C * (rl + 1), :, dlo:dlo + n_d]
                            dst = y3[32 * dz:32 * (dz + 1), r0 + rl, 1:W + 1,
                                     dp_lo:dp_lo + n_d]
                            nc.sync.dma_start(dst, src)
                for dp in range(DC):
                    for rh in range(2):
                        r_out = 1 + 4 * rh
                        pt = psum_p.tile([C, 4, W], FP, name="cps", tag="cps", bufs=6)
                        for ti, (dy, dx) in enumerate(taps):
                            rhs = y3[:, r_out + dy - 1:r_out + dy + 3, dx:dx + W, dp]
                            nc.tensor.matmul(pt[:], lhsT=wst[:, 3 * dy + dx, :],
                                             rhs=rhs, start=(ti == 0), stop=(ti == 8))
                        d_abs = d0 + dp
                        dst = conv_dram[C * (d_abs % 4):C * (d_abs % 4 + 1),
                                        d_abs // 4, 4 * rh:4 * rh + 4, :]
                        stg = pcv.tile([C, 4, W], FP, name="cstg", tag="cstg", bufs=6)
                        nc.vector.tensor_copy(stg[:], pt[:])
                        nc.sync.dma_start(dst, stg[:])

        # =======================================================
        # BN2 stats (stream conv back) + final residual
        # =======================================================
        with tc.tile_pool(name="pf", bufs=1) as pf:
            cd = pf.tile([128, DQ, RPC, W], FP, name="cd", tag="cd")
            nc.sync.dma_start(cd[:], conv_dram[:])
            sAB = pf.tile([128, 2], FP, name="sAB", tag="sAB")
            # Sum absorbs cd's DMA waits on V (multi-wait-capable reduce);
            # the ttr sumsq and the stt below then have them elided.
            nc.vector.tensor_reduce(out=sAB[:, 0:1], in_=cd[:], axis=AX.XYZ, op=ALU.add)
            sqs = pf.tile([128, DQ, RPC, W], FP, name="sqs", tag="sqs")
            cdf = cd[:].rearrange("p a b c -> p (a b c)")
            sqf = sqs[:].rearrange("p a b c -> p (a b c)")
            nc.vector.tensor_tensor_reduce(
                out=sqf, in0=cdf, in1=cdf, scale=1.0, scalar=0.0,
                op0=ALU.mult, op1=ALU.add, accum_out=sAB[:, 1:2],
            )
            bnp2 = psum_p.tile([128, 2], FP, name="bnp2", tag="bnp2")
            nc.tensor.matmul(bnp2[:], lhsT=sel_tiles[1][:], rhs=sAB[:],
                             start=True, stop=True)
            bnst2 = pool_g.tile([128, 2], FP, name="bnst2", tag="bnst2")
            nc.vector.tensor_copy(bnst2[:], bnp2[:])
            nc.sync.dma_start(bn2_in[:], bnst2[:])
            nc.gpsimd.collective_compute(
                "AllReduce", ALU.add, replica_groups=RG,
                ins=[bn2_in.ap().opt()], outs=[bn2_out.ap().opt()],
            )
            sc2, bi2 = bn_scale_bias(bn2_out, bn2g, bn2b, "2")
            rem = pf.tile([128, DQ, RPC, W], FP, name="rem", tag="rem")
            nc.sync.dma_start(rem[:], xr[:])
            # Absorb rem's DMA waits on V with a full-tile copy (sqs is dead
            # scratch by now, after the stats matmul consumed sAB).
            remf = rem[:].rearrange("p a b c -> p (a b c)")
            nc.vector.tensor_copy(sqf, remf)
            # cd = scale2*conv + rem ; out = relu(cd + bias2)  [all V deps]
            half = (DQ * RPC * W) // 2
            nc.vector.scalar_tensor_tensor(
                out=cdf[:, 0:half], in0=cdf[:, 0:half], scalar=sc2[:],
                in1=sqf[:, 0:half], op0=ALU.mult, op1=ALU.add,
            )
            nc.vector.scalar_tensor_tensor(
                out=cdf[:, half:], in0=cdf[:, half:], scalar=sc2[:],
                in1=sqf[:, half:], op0=ALU.mult, op1=ALU.add,
            )
            nc.scalar.activation(cd[:], cd[:], ACT.Relu, bias=bi2[:], scale=1.0)
            nc.sync.dma_start(out_d[:], cd[:])

    return nc


def _get_program():
    if "nc" not in _prog_cache:
        _prog_cache["nc"] = _build_program()
    return _prog_cache["nc"]


def _prep_inputs(x, g, conv_w, bn1_gamma, bn1_beta, bn2_gamma, bn2_beta):
    """Host-side sharding + layout transposes (no arithmetic)."""
    x = np.asarray(x, np.float32)[0]            # [C,D,H,W]
    g = np.asarray(g, np.float32)[0]            # [640,H,W]
    conv_w = np.asarray(conv_w, np.float32)
    ks = g.reshape(4, C, 5, H, W)               # k1..k4

    # conv weight stack: wstk[dz*32+ci, dy*3+dx, co]
    wstk = np.ascontiguousarray(
        conv_w.transpose(2, 1, 3, 4, 0).reshape(3 * C, 9, C)).astype(np.float16)

    # BN masks per h-group (owned local rows are 1..8 of 0..9), columns
    # replicated 4x so the stats matmul yields [128,2] (per-channel stats
    # repeated on every 32-partition block).
    def sel(nrows, r_base):
        m = np.zeros((nrows * C, C), np.float32)
        for rl in range(nrows):
            r = r_base + rl
            if 1 <= r <= 8:
                for c in range(C):
                    m[rl * C + c, c] = 1.0
        return np.ascontiguousarray(np.tile(m, (1, 4)))
    selg0, selg1, selg2 = sel(4, 0), sel(4, 4), sel(2, 8)

    maps = []
    for i in range(NCORES):
        r_lo, r_hi = 8 * i - 1, 8 * i + 9
        xp = np.zeros((RH, C, W, D), np.float32)
        k1p = np.zeros((RH, C, W, 5), np.float32)
        k2p = np.zeros((RH, C, W, 5), np.float32)
        lo, hi = max(r_lo, 0), min(r_hi, H)
        xs = x[:, :, lo:hi, :].transpose(2, 0, 3, 1)     # [r,c,w,d]
        xp[lo - r_lo:hi - r_lo] = xs
        k1p[lo - r_lo:hi - r_lo] = ks[0][:, :, lo:hi, :].transpose(2, 0, 3, 1)
        k2p[lo - r_lo:hi - r_lo] = ks[1][:, :, lo:hi, :].transpose(2, 0, 3, 1)
        cs = slice(CPC * i, CPC * (i + 1))
        xvs = x[:, :, :, cs].transpose(3, 0, 2, 1)       # [col,c,H,D]
        k3s = ks[2][:, :, :, cs].transpose(3, 0, 2, 1)
        k4s = ks[3][:, :, :, cs].transpose(3, 0, 2, 1)
        xrs = x[:, :, 8 * i:8 * (i + 1), :].reshape(C, DQ, 4, RPC, W) \
               .transpose(2, 0, 1, 3, 4).reshape(128, DQ, RPC, W)
        maps.append({
            "xh": np.ascontiguousarray(xp.reshape(RH * C, W, D)),
            "k1h": np.ascontiguousarray(k1p.reshape(RH * C, W, 5)),
            "k2h": np.ascontiguousarray(k2p.reshape(RH * C, W, 5)),
            "xv": np.ascontiguousarray(xvs.reshape(CPC * C, H, D)),
            "k3v": np.ascontiguousarray(k3s.reshape(CPC * C, H, 5)),
            "k4v": np.ascontiguousarray(k4s.reshape(CPC * C, H, 5)),
            "xr": np.ascontiguousarray(xrs),
            "wstk": wstk,
            "bn1g": np.tile(np.asarray(bn1_gamma, np.float32).reshape(C, 1), (4, 1)),
            "bn1b": np.tile(np.asarray(bn1_beta, np.float32).reshape(C, 1), (4, 1)),
            "bn2g": np.tile(np.asarray(bn2_gamma, np.float32).reshape(C, 1), (4, 1)),
            "bn2b": np.tile(np.asarray(bn2_beta, np.float32).reshape(C, 1), (4, 1)),
            "selg0": selg0, "selg1": selg1, "selg2": selg2,
        })
    return maps


def _assemble(results):
    """Per-core out [128, DQ, RPC, W] (dr,c layout) -> full [1,C,D,H,W]."""
    full = np.zeros((1, C, D, H, W), np.float32)
    for i, r in enumerate(results):
        o = np.asarray(r["out"]).reshape(4, C, DQ, RPC, W) \
             .transpose(1, 2, 0, 3, 4).reshape(C, D, RPC, W)
        full[0, :, :, 8 * i:8 * (i + 1), :] = o
    return full


def _run_sim(nc, maps):
    """Fallback: numerically-exact multi-core simulator (no hardware)."""
    from concourse import bass_interp
    sim = bass_interp.MultiCoreSim(nc, NCORES)
    for i in range(NCORES):
        for name, arr in maps[i].items():
            sim.cores[i].tensor(name)[:] = arr
    sim.simulate(check_with_hw=False)
    return [{"out": sim.cores[i].mem_tensor("out")} for i in range(NCORES)]


def kernel(x, g, conv_w, bn1_gamma, bn1_beta, bn2_gamma, bn2_beta):
    from concourse.bass_utils import run_bass_kernel_spmd
    nc = _get_program()
    maps = _prep_inputs(x, g, conv_w, bn1_gamma, bn1_beta, bn2_gamma, bn2_beta)
    try:
        res = run_bass_kernel_spmd(nc, maps, list(range(NCORES)))
        results = res.results
    except Exception as e:  # axon/PJRT unavailable -> simulate
        print(f"kernel: hardware path failed ({type(e).__name__}: {e}); "
              f"falling back to MultiCoreSim", file=sys.stderr)
        results = _run_sim(nc, maps)
    return _assemble(results)


if __name__ == "__main__":
    nc = _build_program()
    print("program built OK; instructions:",
          sum(len(bb.instructions) for bb in nc.main_func.blocks))

